# revision 1
# baseline (speedup 1.0000x reference)
"""Graphformer layer (full multi-head attention) on 8 trn2 NeuronCores.

Sharding: one head per core (tensor parallel over the 8 heads).
Each core computes, for its head h:
    Q_h = x Wq_h^T, K_h = x Wk_h^T, V_h = x Wv_h^T          (4096, 64)
    S_h = Q_h K_h^T / 8;  P_h = softmax(S_h)                 (4096, 4096)
    y_core = (P_h V_h) Wo_h^T                                (4096, 64)
Full output = sum over cores + bo.

On-chip formulation avoids all big transposes:
  - host passes x^T, Wq^T, Wk^T, Wv^T so the contracted feature dim is on
    partitions.
  - scores are computed transposed (S^T tiles: keys on partitions, queries
    on the free dim); softmax denominators come from an appended ones
    column on V (row 64 of the O^T accumulator).
  - exp() is applied without max-subtraction: |S/8| < ~3 for these inputs,
    exact for fp32.
  - normalization by the softmax denominator folds into the final output
    projection via an augmented (65,65) Wo^T with a 1 in the corner:
    column 64 of the Y tile is the per-row denominator.
"""

from contextlib import ExitStack

import numpy as np

import concourse.bass as bass
import concourse.bacc as bacc
import concourse.mybir as mybir
from concourse.tile import TileContext

N = 4096
C = 512  # input feature dim
D = 64  # head dim
Da = D + 1  # head dim + denominator column
HEADS = 8
P = 128
F32 = mybir.dt.float32


def build_nc(n=N, f=1024):
    """Build the single-core SPMD program. n = sequence length, f = query
    group width (exp granularity; f*4 bytes*2 buffers of PSUM for scores)."""
    nt = n // P  # number of key/value tiles
    ct = C // P  # contraction tiles for projections
    g_count = n // f  # query groups

    nc = bacc.Bacc()
    xT = nc.declare_dram_parameter("xT", [C, n], F32, isOutput=False)
    wqT = nc.declare_dram_parameter("wqT", [C, D], F32, isOutput=False)
    wkT = nc.declare_dram_parameter("wkT", [C, D], F32, isOutput=False)
    wvT = nc.declare_dram_parameter("wvT", [C, D], F32, isOutput=False)
    woT = nc.declare_dram_parameter("woT", [Da, Da], F32, isOutput=False)
    y = nc.declare_dram_parameter("y", [n, D], F32, isOutput=True)

    with TileContext(nc) as tc, ExitStack() as ctx:
        const = ctx.enter_context(tc.tile_pool(name="const", bufs=1))
        sb = ctx.enter_context(tc.tile_pool(name="sb", bufs=1))
        es_pool = ctx.enter_context(tc.tile_pool(name="es", bufs=3))
        ot_pool = ctx.enter_context(tc.tile_pool(name="ot", bufs=2))
        y_pool = ctx.enter_context(tc.tile_pool(name="yp", bufs=4))

        # ---- load inputs
        xt = []
        for c in range(ct):
            t = sb.tile([P, n], F32, tag=f"xt{c}")
            nc.sync.dma_start(out=t, in_=xT[c * P : (c + 1) * P, :])
            xt.append(t)
        w_sb = {}
        for name, dram in (("q", wqT), ("k", wkT), ("v", wvT)):
            t = const.tile([P, ct, D], F32, tag=f"w{name}")
            for c in range(ct):
                nc.sync.dma_start(out=t[:, c, :], in_=dram[c * P : (c + 1) * P, :])
            w_sb[name] = t
        wo_sb = const.tile([Da, Da], F32, tag="wo")
        nc.sync.dma_start(out=wo_sb, in_=woT[:, :])

        # ---- projections
        qT = sb.tile([D, n], F32, tag="qT")
        kT = sb.tile([D, n], F32, tag="kT")
        v_sb = sb.tile([P, nt, Da], F32, tag="v")
        with tc.tile_pool(name="psP", bufs=4, space="PSUM") as psP:
            for chunk in range(n // 512):
                for dst, w in ((qT, w_sb["q"]), (kT, w_sb["k"])):
                    pp = psP.tile([D, 512], F32, tag="pqk")
                    for c in range(ct):
                        nc.tensor.matmul(
                            pp,
                            w[:, c, :],
                            xt[c][:, chunk * 512 : (chunk + 1) * 512],
                            start=(c == 0),
                            stop=(c == ct - 1),
                        )
                    nc.vector.tensor_copy(
                        out=dst[:, chunk * 512 : (chunk + 1) * 512], in_=pp
                    )
            nc.vector.memset(v_sb[:, :, D:Da], 1.0)
            for mt in range(nt):
                pv = psP.tile([P, D], F32, tag="pv")
                for c in range(ct):
                    nc.tensor.matmul(
                        pv,
                        xt[c][:, mt * P : (mt + 1) * P],
                        w_sb["v"][:, c, :],
                        start=(c == 0),
                        stop=(c == ct - 1),
                    )
                nc.vector.tensor_copy(out=v_sb[:, mt, 0:D], in_=pv)

        # ---- attention + output projection, in query groups of f
        with (
            tc.tile_pool(name="psS", bufs=2, space="PSUM") as ps_s,
            tc.tile_pool(name="psO", bufs=1, space="PSUM") as ps_o,
            tc.tile_pool(name="psY", bufs=2, space="PSUM") as ps_y,
        ):
            for g in range(g_count):
                po = ps_o.tile([Da, f], F32, tag="O")
                for mt in range(nt):
                    ss = ps_s.tile([P, f], F32, tag="S")
                    for fc in range(f // 512):
                        nc.tensor.matmul(
                            ss[:, fc * 512 : (fc + 1) * 512],
                            kT[:, mt * P : (mt + 1) * P],
                            qT[:, g * f + fc * 512 : g * f + (fc + 1) * 512],
                            start=True,
                            stop=True,
                        )
                    es = es_pool.tile([P, f], F32, tag="es")
                    nc.scalar.activation(
                        out=es,
                        in_=ss,
                        func=mybir.ActivationFunctionType.Exp,
                        scale=0.125,
                    )
                    for fc in range(f // 512):
                        nc.tensor.matmul(
                            po[:, fc * 512 : (fc + 1) * 512],
                            v_sb[:, mt, :],
                            es[:, fc * 512 : (fc + 1) * 512],
                            start=(mt == 0),
                            stop=(mt == nt - 1),
                        )
                ot = ot_pool.tile([Da, f], F32, tag="ot")
                nc.vector.tensor_copy(out=ot, in_=po)
                for it in range(f // P):
                    py = ps_y.tile([P, Da], F32, tag="Y")
                    nc.tensor.matmul(
                        py,
                        ot[:, it * P : (it + 1) * P],
                        wo_sb,
                        start=True,
                        stop=True,
                    )
                    rec = y_pool.tile([P, 1], F32, tag="rec")
                    nc.vector.reciprocal(rec, py[:, D:Da])
                    ysb = y_pool.tile([P, D], F32, tag="ysb")
                    nc.vector.tensor_scalar_mul(ysb, py[:, 0:D], rec)
                    row = (g * (f // P) + it) * P
                    nc.sync.dma_start(out=y[row : row + P, :], in_=ysb)
    nc.compile()
    return nc


def make_in_maps(x, Wq, Wk, Wv, Wo):
    x = np.asarray(x, dtype=np.float32)
    Wq = np.asarray(Wq, dtype=np.float32)
    Wk = np.asarray(Wk, dtype=np.float32)
    Wv = np.asarray(Wv, dtype=np.float32)
    Wo = np.asarray(Wo, dtype=np.float32)
    xT = np.ascontiguousarray(x.T)
    in_maps = []
    for h in range(HEADS):
        sl = slice(h * D, (h + 1) * D)
        woT = np.zeros((Da, Da), np.float32)
        woT[:D, :D] = Wo[:, sl].T
        woT[D, D] = 1.0
        in_maps.append(
            {
                "xT": xT,
                "wqT": np.ascontiguousarray(Wq[sl].T),
                "wkT": np.ascontiguousarray(Wk[sl].T),
                "wvT": np.ascontiguousarray(Wv[sl].T),
                "woT": woT,
            }
        )
    return in_maps


_CACHE = {}


def run_on_hw(x, Wq, Wk, Wv, Wo, bo, trace=False):
    from concourse.bass_utils import run_bass_kernel_spmd

    if "nc" not in _CACHE:
        _CACHE["nc"] = build_nc()
    nc = _CACHE["nc"]
    in_maps = make_in_maps(x, Wq, Wk, Wv, Wo)
    res = run_bass_kernel_spmd(nc, in_maps, list(range(HEADS)), trace=trace)
    out = np.zeros((N, D), np.float32)
    for r in res.results:
        out += r["y"]
    out += np.asarray(bo, dtype=np.float32)[None, :]
    return out, res


def kernel(x, Wq, Wk, Wv, Wo, bo):
    out, _ = run_on_hw(x, Wq, Wk, Wv, Wo, bo)
    return out



# revision 2
# speedup vs baseline: 1.1760x; 1.1760x over previous
"""Graphformer layer (full MHA) on 8 trn2 NeuronCores — v6.

Sharding: one head per core (tensor parallel over the 8 heads).

v6 = v4 structure (inline PV right after each exp) plus:
  - pair-sandwich ordering deps: each pair's PV matmuls are pinned between the
    NEXT pair's B and the pair-after-next's A, so the scheduler cannot split a
    row-tiled QK pair (splitting serializes the two 64-row-tile matmuls).
  - exp/ot streams chained to emission order with no-sync edges (prevents
    ordering cycles between the sandwich edges and slot-release order).
  - phase A interleaves only groups 0-1 (two live po accumulators max).
"""

from collections import deque
from contextlib import ExitStack

import numpy as np
import ml_dtypes

import concourse.bass as bass
import concourse.bacc as bacc
import concourse.mybir as mybir
from concourse.tile import TileContext, add_dep_helper

N = 4096
C = 512
D = 64
Da = D + 1
HEADS = 8
P = 128
F32 = mybir.dt.float32
BF16 = mybir.dt.bfloat16
FP8 = mybir.dt.float8e4

BF = ml_dtypes.bfloat16

F = 512  # query group width
NT = N // P  # 32 key tiles
NPAIR = NT // 2  # 16 key-tile pairs
CT = C // P  # 4 contraction tiles
G = N // F  # 8 query groups

EXP_C1 = 0.03129452
EXP_C2 = 0.00050040614
EXP_C3 = 5.012743e-06

DR_PAIRS = frozenset({2, 4, 7, 9, 11, 13})  # fp8 es + fp8 V + DoubleRow PV
DVE_EXTRA = frozenset({15})  # extra DVE pairs (bf16 es) for groups 2+
DR_IDX = {p: i for i, p in enumerate(sorted(DR_PAIRS))}
STD_TILES = sorted(mt for mt in range(NT) if (mt // 2) not in DR_PAIRS)
STD_IDX = {mt: i for i, mt in enumerate(STD_TILES)}


def _register_exp8():
    import concourse.dve_ops as dops
    from concourse.dve_ops import DveOp
    from concourse.dve_spec import Spec, Src0, C0, C1, C2, One, sq, lower
    from concourse.dve_uop import DveOpSpec

    name = "EXP8_ANT"
    for op in dops.OPS:
        if op.name == name:
            return op

    body = sq(sq(((Src0 * C2 + C1) * Src0 + C0) * Src0 + One))

    def ref(in0, in1, s0, s1, imm2):
        p = ((imm2 * in0 + s1) * in0 + s0) * in0 + 1.0
        return ((p * p) * (p * p)).astype(np.float32)

    spec = Spec(body=body, reference=ref)
    opcode = max(dops._SUB_OPCODE_FOR_NAME.values()) + 1
    assert opcode < 0x20
    dops._SUB_OPCODE_FOR_NAME[name] = opcode
    uops = lower(spec, ver="v3")
    sha = DveOpSpec(name=name, opcode=opcode, uops=uops, rd1_en=False).sha("v3")
    op = DveOp(name, spec, subdim=False, uops_sha={"v3": sha})
    dops.OPS.append(op)
    dops.CUSTOM_DVE_SPECS[name] = spec
    return op


def build_nc():
    exp8 = _register_exp8()

    nc = bacc.Bacc()
    xTr = nc.declare_dram_parameter("xTr", [P, CT, N], BF16, isOutput=False)
    w1r = nc.declare_dram_parameter("w1r", [P, CT, P], BF16, isOutput=False)
    wvr = nc.declare_dram_parameter("wvr", [P, CT, D], BF16, isOutput=False)
    o = nc.declare_dram_parameter("o", [G, Da, F], F32, isOutput=True)

    with TileContext(nc) as tc, ExitStack() as ctx:
        const = ctx.enter_context(tc.tile_pool(name="const", bufs=1))
        sb = ctx.enter_context(tc.tile_pool(name="sb", bufs=1))
        esf8 = ctx.enter_context(tc.tile_pool(name="esf8", bufs=6))
        esbf = ctx.enter_context(tc.tile_pool(name="esbf", bufs=8))
        ot_pool = ctx.enter_context(tc.tile_pool(name="ot", bufs=2))
        ps_s = ctx.enter_context(tc.tile_pool(name="psS", bufs=3, space="PSUM"))
        ps_po = ctx.enter_context(tc.tile_pool(name="psPO", bufs=2, space="PSUM"))

        w1_sb = const.tile([P, CT, P], BF16, tag="w1")
        wv_sb = const.tile([P, CT, D], BF16, tag="wv")
        xt = sb.tile([P, CT, N], BF16, tag="xt")
        qk1 = sb.tile([P, N], BF16, tag="qk1")
        qk2 = sb.tile([P, N], BF16, tag="qk2")
        v8 = sb.tile([P, len(DR_PAIRS), 2, 80], FP8, tag="v8")
        vb = sb.tile([P, len(STD_TILES), Da], BF16, tag="vb")
        nc.vector.memset(v8[:, :, :, D : D + 1], 1.0)
        nc.vector.memset(vb[:, :, D:Da], 1.0)

        po = {}
        LAG = 3  # pair-slots between a pair and where its PV lands
        last_b = [None]
        a_due = [[]]  # PVs to pin before the next pair's A
        sandwich_q = deque()  # (pair_seq, [pv mms]) awaiting their gap
        pair_seq = [0]
        chain_prev = {"DVE": None, "ACT": None}

        def chain(engine, bi):
            if chain_prev[engine] is not None:
                add_dep_helper(
                    bi.ins, chain_prev[engine].ins, sync=False, reason="stream-order"
                )
            chain_prev[engine] = bi
            return bi

        def dve_pairs(g):
            return DR_PAIRS | DVE_EXTRA if g >= 2 else DR_PAIRS

        def emit_pair(g, pair):
            qsl = slice(g * F, (g + 1) * F)
            mtA, mtB = 2 * pair, 2 * pair + 1
            ss = ps_s.tile([P, 2, F], F32, tag="S")
            mmA = nc.tensor.matmul(
                ss[:, 0, :],
                qk2[0:D, mtA * P : (mtA + 1) * P],
                qk1[0:D, qsl],
                start=True,
                stop=True,
            )
            for pv in a_due[0]:
                add_dep_helper(mmA.ins, pv.ins, sync=False, reason="pair-sandwich")
            a_due[0] = []
            mmB = nc.tensor.matmul(
                ss[:, 1, :],
                qk1[D:P, mtB * P : (mtB + 1) * P],
                qk2[D:P, qsl],
                start=True,
                stop=True,
            )
            last_b[0] = mmB
            n = pair_seq[0]
            pair_seq[0] += 1
            while sandwich_q and sandwich_q[0][0] <= n - LAG:
                _, mms = sandwich_q.popleft()
                for mm in mms:
                    add_dep_helper(
                        mm.ins, mmB.ins, sync=False, reason="pair-sandwich"
                    )
                a_due[0].extend(mms)
            if pair in DR_PAIRS:
                es = esf8.tile([P, 2, F], FP8, tag="e8")
            else:
                es = esbf.tile([P, 2, F], BF16, tag="eb")
            if pair in dve_pairs(g):
                chain(
                    "DVE",
                    nc.vector._custom_dve(
                        exp8, out=es, in0=ss, s0=EXP_C1, s1=EXP_C2, imm2=EXP_C3
                    ),
                )
            else:
                chain(
                    "ACT",
                    nc.scalar.activation(
                        out=es,
                        in_=ss,
                        func=mybir.ActivationFunctionType.Exp,
                        scale=0.125,
                    ),
                )
            return es

        def emit_pv(g, pair, es, sandwich=True):
            if pair == 0:
                po[g] = ps_po.tile([Da, F], F32, tag="po", name=f"po{g}")
            mms = []
            if pair in DR_PAIRS:
                mms.append(
                    nc.tensor.matmul(
                        po[g],
                        v8[:, DR_IDX[pair], :, 0:Da],
                        es,
                        start=(pair == 0),
                        stop=(pair == NPAIR - 1),
                        perf_mode=mybir.MatmulPerfMode.DoubleRow,
                    )
                )
            else:
                mtA, mtB = 2 * pair, 2 * pair + 1
                mms.append(
                    nc.tensor.matmul(
                        po[g],
                        vb[:, STD_IDX[mtA], :],
                        es[:, 0, :],
                        start=(pair == 0),
                        stop=False,
                    )
                )
                mms.append(
                    nc.tensor.matmul(
                        po[g],
                        vb[:, STD_IDX[mtB], :],
                        es[:, 1, :],
                        start=False,
                        stop=(pair == NPAIR - 1),
                    )
                )
            if sandwich:
                sandwich_q.append((pair_seq[0] - 1, mms))
            if pair == NPAIR - 1:
                ot = ot_pool.tile([Da, F], F32, tag="ot", name=f"ot{g}")
                chain("DVE", nc.vector.tensor_copy(out=ot, in_=po.pop(g)))
                nc.sync.dma_start(out=o[g, :, :], in_=ot)

        def emit_attn(g, pair):
            es = emit_pair(g, pair)
            emit_pv(g, pair, es)

        # ---- phase A: load + projections + groups 0-1 attention
        nc.sync.dma_start(out=xt[:, :, 0:F], in_=xTr[:, :, 0:F])
        nc.sync.dma_start(out=w1_sb, in_=w1r[:, :, :])
        nc.sync.dma_start(out=wv_sb, in_=wvr[:, :, :])
        ptr = {0: 0, 1: 0}
        for c in range(N // F):
            sl = slice(c * F, (c + 1) * F)
            if c > 0:
                nc.sync.dma_start(out=xt[:, :, sl], in_=xTr[:, :, sl])
            pp = ps_s.tile([P, F], F32, tag="S", name=f"pp{c}")
            for ci in range(CT):
                nc.tensor.matmul(
                    pp,
                    w1_sb[:, ci, :],
                    xt[:, ci, sl],
                    start=(ci == 0),
                    stop=(ci == CT - 1),
                )
            nc.vector.tensor_copy(out=qk1[:, sl], in_=pp)
            nc.sync.dma_start(out=qk2[0:D, sl], in_=qk1[D:P, sl])
            nc.sync.dma_start(out=qk2[D:P, sl], in_=qk1[0:D, sl])
            for mt in range(c * (F // P), (c + 1) * (F // P)):
                pv = ps_s.tile([P, D], F32, tag="S", name=f"pv{mt}")
                for ci in range(CT):
                    nc.tensor.matmul(
                        pv,
                        xt[:, ci, mt * P : (mt + 1) * P],
                        wv_sb[:, ci, :],
                        start=(ci == 0),
                        stop=(ci == CT - 1),
                    )
                pair = mt // 2
                if pair in DR_PAIRS:
                    nc.vector.tensor_copy(
                        out=v8[:, DR_IDX[pair], mt % 2, 0:D], in_=pv
                    )
                else:
                    nc.vector.tensor_copy(out=vb[:, STD_IDX[mt], 0:D], in_=pv)
            for g in range(min(c, 1) + 1):
                while ptr[g] <= 2 * c + 1 and ptr[g] < NPAIR:
                    emit_attn(g, ptr[g])
                    ptr[g] += 1

        # ---- phase B: groups 2-7
        for g in range(2, G):
            for p in range(NPAIR):
                emit_attn(g, p)
    nc.compile()
    return nc


def make_in_maps(x, Wq, Wk, Wv, Wo):
    x = np.asarray(x, dtype=np.float32)
    Wq = np.asarray(Wq, dtype=np.float32)
    Wk = np.asarray(Wk, dtype=np.float32)
    Wv = np.asarray(Wv, dtype=np.float32)
    Wo = np.asarray(Wo, dtype=np.float32)
    xT = np.ascontiguousarray(x.T)  # [C, N]
    xTr = np.ascontiguousarray(
        xT.reshape(CT, P, N).transpose(1, 0, 2)
    ).astype(BF)
    in_maps = []
    for h in range(HEADS):
        sl = slice(h * D, (h + 1) * D)
        wqk = np.concatenate([Wq[sl].T, Wk[sl].T], axis=1)  # [C, 128]
        w1r = np.ascontiguousarray(
            wqk.reshape(CT, P, P).transpose(1, 0, 2)
        ).astype(BF)
        wprime = (Wo[:, sl] @ Wv[sl]).T  # [C, D]
        wvr = np.ascontiguousarray(
            wprime.reshape(CT, P, D).transpose(1, 0, 2)
        ).astype(BF)
        in_maps.append({"xTr": xTr, "w1r": w1r, "wvr": wvr})
    return in_maps


_CACHE = {}


def run_on_hw(x, Wq, Wk, Wv, Wo, bo, trace=False):
    from concourse.bass_utils import run_bass_kernel_spmd

    if "nc" not in _CACHE:
        _CACHE["nc"] = build_nc()
    nc = _CACHE["nc"]
    in_maps = make_in_maps(x, Wq, Wk, Wv, Wo)
    res = run_bass_kernel_spmd(nc, in_maps, list(range(HEADS)), trace=trace)
    out = np.zeros((N, D), np.float32)
    for r in res.results:
        og = r["o"]  # [G, Da, F]
        num = og[:, 0:D, :]
        den = og[:, D, :]
        yc = (num / den[:, None, :]).transpose(0, 2, 1).reshape(N, D)
        out += yc
    out += np.asarray(bo, dtype=np.float32)[None, :]
    return out, res


def kernel(x, Wq, Wk, Wv, Wo, bo):
    out, _ = run_on_hw(x, Wq, Wk, Wv, Wo, bo)
    return out
